# revision 23
# baseline (speedup 1.0000x reference)
"""Euler integrator (low-rank quadratic Christoffel term) on 8 trn2 NeuronCores.

Math: per step   h = v @ U; gamma = (h*h) @ W; v' = v + dt*(force - gamma);
                 x' = wrap(x + dt*v)
Reduction: dynamics close in the rank-64 space:
    h_{t+1} = h_t + dt*(force@U) - (h_t^2) @ (dt*W@U)
    v_T = v_0 + T*dt*force - dt * A @ W,          A = sum_t h_t^2
    x_T = wrap(x_0 + T*dt*v_0 + 28*dt^2*force - dt^2 * G @ W),
                                                  G = sum_t (T-1-t) h_t^2
with T=8.  The dt*(force@U) term inside the h recursion is O(1e-3) relative
to h and is dropped (adds ~5e-4 rel err; budget is 2e-2) — this removes all
force transposes and the per-step fU-add matmuls.

v2 layout/engine plan (trace-driven):
  - loads/stores use "(p n) d" packing: one contiguous 8KB chunk per
    partition per 1MB DMA (vs 8x1KB strided lines before).
  - v is transposed on the TensorEngine (is_transpose matmuls into bf16
    PSUM) instead of DMA xbar transposes (1.23us/block on the Sync engine
    -- was 1.26ms/core, the old bottleneck).
  - h update uses one block-diagonal matmul per step (both 64-partition
    halves at once) instead of four 64-wide matmuls.
  - epilogue identity-matmuls add the bf16 force/v0 terms in PSUM (baseline
    numerics); exact fp32 x0/v0 adds happen on DVE; the wrap round-subtract
    runs on GpSimd to offload DVE.
HBM traffic is the 5-tensor roofline: 160MB/core ~ 450us at 358GB/s.
"""

import sys

sys.path.insert(0, "/opt/trn_rl_repo")

import numpy as np
import ml_dtypes

import concourse.bacc as bacc
import concourse.mybir as mybir
import concourse.tile as tile
from concourse.tile_rust import add_dep_helper
from concourse.bass_utils import run_bass_kernel_spmd

F32 = mybir.dt.float32
BF16 = mybir.dt.bfloat16

DT = 0.01
PI = float(np.pi)
TWO_PI = 2.0 * PI
B, D, R = 262144, 256, 64
NCORES = 8
BL = B // NCORES          # rows per core
STEPS = 8
PACK = 1024               # batch rows per pack
NBLK = PACK // 128        # natural 128-row blocks per pack (8)
HN = 512                  # free size of h-space tiles (PACK/2)
MAGIC = 12582912.0        # 1.5 * 2**23 (fp32 RNE rounding trick)


def _chain(*insts):
    for a, b in zip(insts[1:], insts[:-1]):
        add_dep_helper(a.ins, b.ins, sync=True, reason="psum group order")


def _build(bl: int):
    npack = bl // PACK
    nc = bacc.Bacc("TRN2", target_bir_lowering=False, debug=False)

    xg = nc.declare_dram_parameter("xg", [bl, D], F32, isOutput=False)
    vg = nc.declare_dram_parameter("vg", [bl, D], F32, isOutput=False)
    fg = nc.declare_dram_parameter("fg", [bl, D], F32, isOutput=False)
    # constants (host-prepared, tiny; all bf16 for single-pass matmuls)
    cdefs = {
        "u0z": 128, "u1z": 128,     # [U0|0], [U1|0]
        "u0": R, "u1": R,           # U halves
        "mdn2": 128,                # blockdiag(-dt*(W@U), same)
        "wn": D, "wnn": D,          # -dt*W, -dt^2*W, dup'd on both halves
        "i128": 128,                # I_128 (A accumulation + transposes)
        "if8": 128, "i28": 128,     # 8dt*I, 28dt^2*I
    }
    cdram = {
        nm: nc.declare_dram_parameter(nm, [128, w], BF16, isOutput=False)
        for nm, w in cdefs.items()
    }
    xo = nc.declare_dram_parameter("xo", [bl, D], F32, isOutput=True)
    vo = nc.declare_dram_parameter("vo", [bl, D], F32, isOutput=True)

    A = mybir.AluOpType

    with tile.TileContext(nc) as tc:
        with (
            tc.tile_pool(name="consts", bufs=1) as cpool,
            tc.tile_pool(name="nat", bufs=4) as nat,
            tc.tile_pool(name="natx", bufs=4) as natx,
            tc.tile_pool(name="natb", bufs=2) as natb,
            tc.tile_pool(name="trans", bufs=3) as trans,
            tc.tile_pool(name="hsp", bufs=4) as hsp,
            tc.tile_pool(name="acc", bufs=2) as accp,
            tc.tile_pool(name="atp", bufs=2) as atp,
            tc.tile_pool(name="outp", bufs=2) as outp,
            tc.tile_pool(name="wrapp", bufs=2) as wrapp,
            tc.tile_pool(name="ptr", bufs=1, space="PSUM") as ptrp,
            tc.tile_pool(name="ph", bufs=3, space="PSUM") as php,
            tc.tile_pool(name="pA", bufs=1, space="PSUM") as pAp,
            tc.tile_pool(name="pe", bufs=1, space="PSUM") as pep,
        ):
            cs = {}
            for nm, w in cdefs.items():
                t_ = cpool.tile([128, w], BF16, tag=nm)
                # scalar HWDGE ring: don't head-of-line-block pack loads on sync
                nc.scalar.dma_start(out=t_[:], in_=cdram[nm][:])
                cs[nm] = t_
            magic_s = cpool.tile([128, 1], F32, tag="magic")
            nc.vector.memset(magic_s[:], MAGIC)
            nmagic_s = cpool.tile([128, 1], F32, tag="nmagic")
            nc.vector.memset(nmagic_s[:], -MAGIC)

            for p in range(npack):
                rows = slice(p * PACK, (p + 1) * PACK)

                # ---- load x, v natural fp32 (contiguous 8KB/partition);
                #      force is only ever consumed in bf16, so cast it
                #      during the DMA (SWDGE) and never load it in fp32.
                vt = nat.tile([128, NBLK, D], F32, tag="vt")
                xt = natx.tile([128, NBLK, D], F32, tag="xt")
                fb = natb.tile([128, NBLK, D], BF16, tag="fb")
                nc.sync.dma_start(
                    out=vt[:], in_=vg[rows, :].rearrange("(p n) d -> p n d", p=128)
                )
                nc.gpsimd.dma_start(
                    out=fb[:], in_=fg[rows, :].rearrange("(p n) d -> p n d", p=128)
                )
                nc.sync.dma_start(
                    out=xt[:], in_=xg[rows, :].rearrange("(p n) d -> p n d", p=128)
                )

                # ---- cast v to bf16 (ACT)
                vb = natb.tile([128, NBLK, D], BF16, tag="vb")
                nc.scalar.copy(vb[:], vt[:])

                # ---- transpose v on PE into bf16 PSUM, copy to SBUF (DVE)
                ptr0 = ptrp.tile([128, PACK], BF16, tag="ptr0")
                ptr1 = ptrp.tile([128, PACK], BF16, tag="ptr1")
                for dch, ptr in ((0, ptr0), (1, ptr1)):
                    tr = []
                    for n in range(NBLK):
                        tr.append(nc.tensor.transpose(
                            ptr[:, n * 128:(n + 1) * 128],
                            vb[:, n, dch * 128:(dch + 1) * 128],
                            cs["i128"][:],
                        ))
                    _chain(*tr)
                vT0 = trans.tile([128, PACK], BF16, tag="vT0")
                vT1 = trans.tile([128, PACK], BF16, tag="vT1")
                nc.vector.tensor_copy(vT0[:], ptr0[:])
                nc.vector.tensor_copy(vT1[:], ptr1[:])

                # ---- h0 into persistent psum bank
                ph = php.tile([128, HN], F32, tag="ph")
                _chain(
                    nc.tensor.matmul(
                        ph[:, :], cs["u0z"][:], vT0[:, 0:HN],
                        start=True, stop=False,
                    ),
                    nc.tensor.matmul(
                        ph[64:128, :], cs["u0"][:], vT0[:, HN:PACK],
                        start=False, stop=False, skip_group_check=True,
                    ),
                    nc.tensor.matmul(
                        ph[64:128, :], cs["u1"][:], vT1[:, HN:PACK],
                        start=False, stop=False, skip_group_check=True,
                    ),
                    nc.tensor.matmul(
                        ph[:, :], cs["u1z"][:], vT1[:, 0:HN],
                        start=False, stop=True,
                    ),
                )

                # ---- step loop: squares on ACT, A in PSUM via identity MMs,
                #      G via fused DVE stt, h updated by one blockdiag MM/step
                pA = pAp.tile([128, HN], F32, tag="pA")
                Gacc = accp.tile([128, HN], BF16, tag="Gacc")
                a_mms = []
                for t in range(STEPS):
                    hsq = hsp.tile([128, HN], BF16, tag="hsq")
                    nc.scalar.square(hsq[:], ph[:])
                    # critical-path h update FIRST: the next square waits on it,
                    # while the A matmul and G ops have a whole step of slack
                    if t < STEPS - 1:
                        nc.tensor.matmul(
                            ph[:, :], cs["mdn2"][:], hsq[:],
                            start=False, stop=False, skip_group_check=True,
                        )
                    a_mms.append(nc.tensor.matmul(
                        pA[:, :], cs["i128"][:], hsq[:],
                        start=(t == 0), stop=(t == STEPS - 1),
                    ))
                    if t == 0:
                        nc.vector.tensor_scalar(
                            Gacc[:], hsq[:], float(STEPS - 1), None, A.mult,
                        )
                    elif t <= STEPS - 2:
                        nc.vector.scalar_tensor_tensor(
                            out=Gacc[:], in0=hsq[:],
                            scalar=float(STEPS - 1 - t),
                            in1=Gacc[:], op0=A.mult, op1=A.add,
                        )
                _chain(*a_mms)
                At = atp.tile([128, HN], BF16, tag="At")
                nc.scalar.copy(At[:], pA[:])

                # ---- epilogue
                vf_sb = outp.tile([128, NBLK, D], F32, tag="vf_sb")
                xf_sb = outp.tile([128, NBLK, D], F32, tag="xf_sb")

                for bg in range(4):      # bank groups: 2 natural blocks each
                    b0, b1 = bg * 2, bg * 2 + 2
                    pvf = pep.tile([128, 2, D], F32, tag="pvf")
                    pxf = pep.tile([128, 2, D], F32, tag="pxf")
                    vf_mms = []
                    xf_mms = []
                    for j in range(2):
                        blk = bg * 2 + j
                        half = blk // 4
                        hsl = slice(half * 64, (half + 1) * 64)
                        lsl = slice((blk % 4) * 128, (blk % 4) * 128 + 128)
                        vf_mms.append(nc.tensor.matmul(
                            pvf[:, j, :], At[hsl, lsl], cs["wn"][hsl, :],
                            start=(j == 0), stop=False,
                        ))
                        xf_mms.append(nc.tensor.matmul(
                            pxf[:, j, :], Gacc[hsl, lsl], cs["wnn"][hsl, :],
                            start=(j == 0), stop=False,
                        ))
                    vf_mms.append(nc.tensor.matmul(
                        pvf[:, :, :], cs["if8"][:], fb[:, b0:b1, :],
                        start=False, stop=True,
                    ))
                    xf_mms.append(nc.tensor.matmul(
                        pxf[:, :, :], cs["if8"][:], vb[:, b0:b1, :],
                        start=False, stop=False,
                    ))
                    xf_mms.append(nc.tensor.matmul(
                        pxf[:, :, :], cs["i28"][:], fb[:, b0:b1, :],
                        start=False, stop=True,
                    ))
                    _chain(*vf_mms)
                    _chain(*xf_mms)

                    # vf = v0 + (8dt*force + A@Wn)     [exact fp32 add, DVE]
                    nc.vector.tensor_tensor(
                        vf_sb[:, b0:b1, :], vt[:, b0:b1, :], pvf[:], A.add
                    )
                    # q = x0 + (8dt*v0 + 28dt^2*force + G@Wnn)
                    q = wrapp.tile([128, 2, D], F32, tag="q")
                    nc.vector.tensor_tensor(q[:], xt[:, b0:b1, :], pxf[:], A.add)
                    # wrap: r = RNE(q/2pi) via magic const; xf = q - 2pi*r
                    a1 = wrapp.tile([128, 2, D], F32, tag="a1")
                    nc.scalar.activation(
                        out=a1[:], in_=q[:],
                        func=mybir.ActivationFunctionType.Identity,
                        bias=magic_s[:], scale=1.0 / TWO_PI,
                    )
                    rr = wrapp.tile([128, 2, D], F32, tag="rr")
                    nc.scalar.activation(
                        out=rr[:], in_=a1[:],
                        func=mybir.ActivationFunctionType.Identity,
                        bias=nmagic_s[:], scale=1.0,
                    )
                    nc.vector.scalar_tensor_tensor(
                        out=xf_sb[:, b0:b1, :], in0=rr[:],
                        scalar=-TWO_PI, in1=q[:], op0=A.mult, op1=A.add,
                    )

                # stores on the SWDGE ring (GpSimd is idle): loads on the sync
                # HWDGE ring never queue behind them
                nc.gpsimd.dma_start(
                    out=vo[rows, :].rearrange("(p n) d -> p n d", p=128),
                    in_=vf_sb[:],
                )
                nc.gpsimd.dma_start(
                    out=xo[rows, :].rearrange("(p n) d -> p n d", p=128),
                    in_=xf_sb[:],
                )

    nc.compile()
    return nc


_NC_CACHE = {}


def _get_nc(bl: int):
    if bl not in _NC_CACHE:
        _NC_CACHE[bl] = _build(bl)
    return _NC_CACHE[bl]


def _consts(U, W):
    U32 = np.ascontiguousarray(U, dtype=np.float32)
    W32 = np.ascontiguousarray(W, dtype=np.float32)
    bf = ml_dtypes.bfloat16
    dup = lambda a: np.concatenate([a, a], axis=0)
    md = -(DT * (W32 @ U32))
    eye = np.eye(128, dtype=np.float32)
    z = np.zeros((128, 64), np.float32)
    mdn2 = np.zeros((128, 128), np.float32)
    mdn2[:64, :64] = md
    mdn2[64:, 64:] = md
    return {
        "u0z": np.concatenate([U32[:128, :], z], axis=1).astype(bf),
        "u1z": np.concatenate([U32[128:, :], z], axis=1).astype(bf),
        "u0": U32[:128, :].astype(bf),
        "u1": U32[128:, :].astype(bf),
        "mdn2": mdn2.astype(bf),
        "wn": dup(-DT * W32).astype(bf),
        "wnn": dup(-DT * DT * W32).astype(bf),
        "i128": eye.astype(bf),
        "if8": ((8.0 * DT) * eye).astype(bf),
        "i28": ((28.0 * DT * DT) * eye).astype(bf),
    }


def kernel(x, v, force, U, W, steps=STEPS, **_ignored):
    assert int(steps) == STEPS, f"kernel hardcodes steps={STEPS}, got {steps}"
    x = np.ascontiguousarray(x, dtype=np.float32)
    v = np.ascontiguousarray(v, dtype=np.float32)
    force = np.ascontiguousarray(force, dtype=np.float32)
    consts = _consts(U, W)

    nc = _get_nc(BL)
    in_maps = []
    for i in range(NCORES):
        sl = slice(i * BL, (i + 1) * BL)
        m = {"xg": x[sl], "vg": v[sl], "fg": force[sl]}
        m.update(consts)
        in_maps.append(m)

    res = run_bass_kernel_spmd(nc, in_maps, core_ids=list(range(NCORES)))
    xf = np.concatenate([res.results[i]["xo"] for i in range(NCORES)], axis=0)
    vf = np.concatenate([res.results[i]["vo"] for i in range(NCORES)], axis=0)
    return (xf, vf)


# revision 24
# speedup vs baseline: 1.1322x; 1.1322x over previous
"""Euler integrator (low-rank quadratic Christoffel term) on 8 trn2 NeuronCores.

Math: per step   h = v @ U; gamma = (h*h) @ W; v' = v + dt*(force - gamma);
                 x' = wrap(x + dt*v)
Reduction: dynamics close in the rank-64 space:
    h_{t+1} = h_t + dt*(force@U) - (h_t^2) @ (dt*W@U)
    v_T = v_0 + T*dt*force - dt * A @ W,          A = sum_t h_t^2
    x_T = wrap(x_0 + T*dt*v_0 + 28*dt^2*force - dt^2 * G @ W),
                                                  G = sum_t (T-1-t) h_t^2
with T=8.  The dt*(force@U) term inside the h recursion is O(1e-3) relative
to h and is dropped (adds ~5e-4 rel err; budget is 2e-2) — this removes all
force transposes and the per-step fU-add matmuls.

v2 layout/engine plan (trace-driven):
  - loads/stores use "(p n) d" packing: one contiguous 8KB chunk per
    partition per 1MB DMA (vs 8x1KB strided lines before).
  - v is transposed on the TensorEngine (is_transpose matmuls into bf16
    PSUM) instead of DMA xbar transposes (1.23us/block on the Sync engine
    -- was 1.26ms/core, the old bottleneck).
  - h update uses one block-diagonal matmul per step (both 64-partition
    halves at once) instead of four 64-wide matmuls.
  - epilogue identity-matmuls add the bf16 force/v0 terms in PSUM (baseline
    numerics); exact fp32 x0/v0 adds happen on DVE; the wrap round-subtract
    runs on GpSimd to offload DVE.
HBM traffic is the 5-tensor roofline: 160MB/core ~ 450us at 358GB/s.
"""

import sys

sys.path.insert(0, "/opt/trn_rl_repo")

import numpy as np
import ml_dtypes

import concourse.bacc as bacc
import concourse.mybir as mybir
import concourse.tile as tile
from concourse.tile_rust import add_dep_helper
from concourse.bass_utils import run_bass_kernel_spmd

F32 = mybir.dt.float32
BF16 = mybir.dt.bfloat16

DT = 0.01
PI = float(np.pi)
TWO_PI = 2.0 * PI
B, D, R = 262144, 256, 64
NCORES = 8
BL = B // NCORES          # rows per core
STEPS = 8
PACK = 1024               # batch rows per pack
NBLK = PACK // 128        # natural 128-row blocks per pack (8)
HN = 512                  # free size of h-space tiles (PACK/2)
MAGIC = 12582912.0        # 1.5 * 2**23 (fp32 RNE rounding trick)


def _chain(*insts):
    for a, b in zip(insts[1:], insts[:-1]):
        add_dep_helper(a.ins, b.ins, sync=True, reason="psum group order")


def _build(bl: int):
    npack = bl // PACK
    nc = bacc.Bacc("TRN2", target_bir_lowering=False, debug=False)

    xg = nc.declare_dram_parameter("xg", [bl, D], F32, isOutput=False)
    vg = nc.declare_dram_parameter("vg", [bl, D], F32, isOutput=False)
    fg = nc.declare_dram_parameter("fg", [bl, D], F32, isOutput=False)
    # constants (host-prepared, tiny; all bf16 for single-pass matmuls)
    cdefs = {
        "u0z": 128, "u1z": 128,     # [U0|0], [U1|0]
        "u0": R, "u1": R,           # U halves
        "mdn2": 128,                # blockdiag(-dt*(W@U), same)
        "wn": D, "wnn": D,          # -dt*W, -dt^2*W, dup'd on both halves
        "i128": 128,                # I_128 (A accumulation + transposes)
        "if8": 128, "i28": 128,     # 8dt*I, 28dt^2*I
    }
    cdram = {
        nm: nc.declare_dram_parameter(nm, [128, w], BF16, isOutput=False)
        for nm, w in cdefs.items()
    }
    xo = nc.declare_dram_parameter("xo", [bl, D], F32, isOutput=True)
    vo = nc.declare_dram_parameter("vo", [bl, D], F32, isOutput=True)

    A = mybir.AluOpType

    with tile.TileContext(nc) as tc:
        with (
            tc.tile_pool(name="consts", bufs=1) as cpool,
            tc.tile_pool(name="nat", bufs=3) as nat,
            tc.tile_pool(name="natx", bufs=3) as natx,
            tc.tile_pool(name="natb", bufs=2) as natb,
            tc.tile_pool(name="trans", bufs=2) as trans,
            tc.tile_pool(name="hsp", bufs=3) as hsp,
            tc.tile_pool(name="acc", bufs=2) as accp,
            tc.tile_pool(name="atp", bufs=2) as atp,
            tc.tile_pool(name="outp", bufs=2) as outp,
            tc.tile_pool(name="wrapp", bufs=2) as wrapp,
            tc.tile_pool(name="ptr", bufs=1, space="PSUM") as ptrp,
            tc.tile_pool(name="ph", bufs=3, space="PSUM") as php,
            tc.tile_pool(name="pA", bufs=1, space="PSUM") as pAp,
            tc.tile_pool(name="pe", bufs=1, space="PSUM") as pep,
        ):
            cs = {}
            for nm, w in cdefs.items():
                t_ = cpool.tile([128, w], BF16, tag=nm)
                # scalar HWDGE ring: don't head-of-line-block pack loads on sync
                nc.scalar.dma_start(out=t_[:], in_=cdram[nm][:])
                cs[nm] = t_
            magic_s = cpool.tile([128, 1], F32, tag="magic")
            nc.vector.memset(magic_s[:], MAGIC)
            nmagic_s = cpool.tile([128, 1], F32, tag="nmagic")
            nc.vector.memset(nmagic_s[:], -MAGIC)

            for p in range(npack):
                rows = slice(p * PACK, (p + 1) * PACK)

                # ---- load x, v natural fp32 (contiguous 8KB/partition);
                #      force is only ever consumed in bf16, so cast it
                #      during the DMA (SWDGE) and never load it in fp32.
                vt = nat.tile([128, NBLK, D], F32, tag="vt")
                xt = natx.tile([128, NBLK, D], F32, tag="xt")
                fb = natb.tile([128, NBLK, D], BF16, tag="fb")
                nc.sync.dma_start(
                    out=vt[:], in_=vg[rows, :].rearrange("(p n) d -> p n d", p=128)
                )
                nc.gpsimd.dma_start(
                    out=fb[:], in_=fg[rows, :].rearrange("(p n) d -> p n d", p=128)
                )
                nc.sync.dma_start(
                    out=xt[:], in_=xg[rows, :].rearrange("(p n) d -> p n d", p=128)
                )

                # ---- cast v to bf16 (ACT)
                vb = natb.tile([128, NBLK, D], BF16, tag="vb")
                nc.scalar.copy(vb[:], vt[:])

                # ---- transpose v on PE into bf16 PSUM, copy to SBUF (DVE)
                ptr0 = ptrp.tile([128, PACK], BF16, tag="ptr0")
                ptr1 = ptrp.tile([128, PACK], BF16, tag="ptr1")
                for dch, ptr in ((0, ptr0), (1, ptr1)):
                    tr = []
                    for n in range(NBLK):
                        tr.append(nc.tensor.transpose(
                            ptr[:, n * 128:(n + 1) * 128],
                            vb[:, n, dch * 128:(dch + 1) * 128],
                            cs["i128"][:],
                        ))
                    _chain(*tr)
                vT0 = trans.tile([128, PACK], BF16, tag="vT0")
                vT1 = trans.tile([128, PACK], BF16, tag="vT1")
                nc.vector.tensor_copy(vT0[:], ptr0[:])
                nc.vector.tensor_copy(vT1[:], ptr1[:])

                # ---- h0 into persistent psum bank
                ph = php.tile([128, HN], F32, tag="ph")
                _chain(
                    nc.tensor.matmul(
                        ph[:, :], cs["u0z"][:], vT0[:, 0:HN],
                        start=True, stop=False,
                    ),
                    nc.tensor.matmul(
                        ph[64:128, :], cs["u0"][:], vT0[:, HN:PACK],
                        start=False, stop=False, skip_group_check=True,
                    ),
                    nc.tensor.matmul(
                        ph[64:128, :], cs["u1"][:], vT1[:, HN:PACK],
                        start=False, stop=False, skip_group_check=True,
                    ),
                    nc.tensor.matmul(
                        ph[:, :], cs["u1z"][:], vT1[:, 0:HN],
                        start=False, stop=True,
                    ),
                )

                # ---- step loop: squares on ACT, A in PSUM via identity MMs,
                #      G via fused DVE stt, h updated by one blockdiag MM/step
                pA = pAp.tile([128, HN], F32, tag="pA")
                Gacc = accp.tile([128, HN], BF16, tag="Gacc")
                a_mms = []
                for t in range(STEPS):
                    hsq = hsp.tile([128, HN], BF16, tag="hsq")
                    nc.scalar.square(hsq[:], ph[:])
                    # critical-path h update FIRST: the next square waits on it,
                    # while the A matmul and G ops have a whole step of slack
                    if t < STEPS - 1:
                        nc.tensor.matmul(
                            ph[:, :], cs["mdn2"][:], hsq[:],
                            start=False, stop=False, skip_group_check=True,
                        )
                    a_mms.append(nc.tensor.matmul(
                        pA[:, :], cs["i128"][:], hsq[:],
                        start=(t == 0), stop=(t == STEPS - 1),
                    ))
                    if t == 0:
                        nc.vector.tensor_scalar(
                            Gacc[:], hsq[:], float(STEPS - 1), None, A.mult,
                        )
                    elif t <= STEPS - 2:
                        nc.vector.scalar_tensor_tensor(
                            out=Gacc[:], in0=hsq[:],
                            scalar=float(STEPS - 1 - t),
                            in1=Gacc[:], op0=A.mult, op1=A.add,
                        )
                _chain(*a_mms)
                At = atp.tile([128, HN], BF16, tag="At")
                nc.scalar.copy(At[:], pA[:])

                # ---- epilogue
                vf_sb = outp.tile([128, NBLK, D], F32, tag="vf_sb")
                xf_sb = outp.tile([128, NBLK, D], F32, tag="xf_sb")

                for bg in range(4):      # bank groups: 2 natural blocks each
                    b0, b1 = bg * 2, bg * 2 + 2
                    pvf = pep.tile([128, 2, D], F32, tag="pvf")
                    pxf = pep.tile([128, 2, D], F32, tag="pxf")
                    vf_mms = []
                    xf_mms = []
                    for j in range(2):
                        blk = bg * 2 + j
                        half = blk // 4
                        hsl = slice(half * 64, (half + 1) * 64)
                        lsl = slice((blk % 4) * 128, (blk % 4) * 128 + 128)
                        vf_mms.append(nc.tensor.matmul(
                            pvf[:, j, :], At[hsl, lsl], cs["wn"][hsl, :],
                            start=(j == 0), stop=False,
                        ))
                        xf_mms.append(nc.tensor.matmul(
                            pxf[:, j, :], Gacc[hsl, lsl], cs["wnn"][hsl, :],
                            start=(j == 0), stop=False,
                        ))
                    vf_mms.append(nc.tensor.matmul(
                        pvf[:, :, :], cs["if8"][:], fb[:, b0:b1, :],
                        start=False, stop=True,
                    ))
                    xf_mms.append(nc.tensor.matmul(
                        pxf[:, :, :], cs["if8"][:], vb[:, b0:b1, :],
                        start=False, stop=False,
                    ))
                    xf_mms.append(nc.tensor.matmul(
                        pxf[:, :, :], cs["i28"][:], fb[:, b0:b1, :],
                        start=False, stop=True,
                    ))
                    _chain(*vf_mms)
                    _chain(*xf_mms)

                    # vf = v0 + (8dt*force + A@Wn)     [exact fp32 add, DVE]
                    nc.vector.tensor_tensor(
                        vf_sb[:, b0:b1, :], vt[:, b0:b1, :], pvf[:], A.add
                    )
                    # q = x0 + (8dt*v0 + 28dt^2*force + G@Wnn)
                    q = wrapp.tile([128, 2, D], F32, tag="q")
                    nc.vector.tensor_tensor(q[:], xt[:, b0:b1, :], pxf[:], A.add)
                    # wrap: r = RNE(q/2pi) via magic const; xf = q - 2pi*r
                    a1 = wrapp.tile([128, 2, D], F32, tag="a1")
                    nc.scalar.activation(
                        out=a1[:], in_=q[:],
                        func=mybir.ActivationFunctionType.Identity,
                        bias=magic_s[:], scale=1.0 / TWO_PI,
                    )
                    rr = wrapp.tile([128, 2, D], F32, tag="rr")
                    nc.scalar.activation(
                        out=rr[:], in_=a1[:],
                        func=mybir.ActivationFunctionType.Identity,
                        bias=nmagic_s[:], scale=1.0,
                    )
                    nc.vector.scalar_tensor_tensor(
                        out=xf_sb[:, b0:b1, :], in0=rr[:],
                        scalar=-TWO_PI, in1=q[:], op0=A.mult, op1=A.add,
                    )

                # stores on the SWDGE ring (GpSimd is idle): loads on the sync
                # HWDGE ring never queue behind them
                nc.gpsimd.dma_start(
                    out=vo[rows, :].rearrange("(p n) d -> p n d", p=128),
                    in_=vf_sb[:],
                )
                nc.gpsimd.dma_start(
                    out=xo[rows, :].rearrange("(p n) d -> p n d", p=128),
                    in_=xf_sb[:],
                )

    nc.compile()
    return nc


_NC_CACHE = {}


def _get_nc(bl: int):
    if bl not in _NC_CACHE:
        _NC_CACHE[bl] = _build(bl)
    return _NC_CACHE[bl]


def _consts(U, W):
    U32 = np.ascontiguousarray(U, dtype=np.float32)
    W32 = np.ascontiguousarray(W, dtype=np.float32)
    bf = ml_dtypes.bfloat16
    dup = lambda a: np.concatenate([a, a], axis=0)
    md = -(DT * (W32 @ U32))
    eye = np.eye(128, dtype=np.float32)
    z = np.zeros((128, 64), np.float32)
    mdn2 = np.zeros((128, 128), np.float32)
    mdn2[:64, :64] = md
    mdn2[64:, 64:] = md
    return {
        "u0z": np.concatenate([U32[:128, :], z], axis=1).astype(bf),
        "u1z": np.concatenate([U32[128:, :], z], axis=1).astype(bf),
        "u0": U32[:128, :].astype(bf),
        "u1": U32[128:, :].astype(bf),
        "mdn2": mdn2.astype(bf),
        "wn": dup(-DT * W32).astype(bf),
        "wnn": dup(-DT * DT * W32).astype(bf),
        "i128": eye.astype(bf),
        "if8": ((8.0 * DT) * eye).astype(bf),
        "i28": ((28.0 * DT * DT) * eye).astype(bf),
    }


def kernel(x, v, force, U, W, steps=STEPS, **_ignored):
    assert int(steps) == STEPS, f"kernel hardcodes steps={STEPS}, got {steps}"
    x = np.ascontiguousarray(x, dtype=np.float32)
    v = np.ascontiguousarray(v, dtype=np.float32)
    force = np.ascontiguousarray(force, dtype=np.float32)
    consts = _consts(U, W)

    nc = _get_nc(BL)
    in_maps = []
    for i in range(NCORES):
        sl = slice(i * BL, (i + 1) * BL)
        m = {"xg": x[sl], "vg": v[sl], "fg": force[sl]}
        m.update(consts)
        in_maps.append(m)

    res = run_bass_kernel_spmd(nc, in_maps, core_ids=list(range(NCORES)))
    xf = np.concatenate([res.results[i]["xo"] for i in range(NCORES)], axis=0)
    vf = np.concatenate([res.results[i]["vo"] for i in range(NCORES)], axis=0)
    return (xf, vf)
